# revision 19
# baseline (speedup 1.0000x reference)
"""Trainium2 Bass kernel for ConvMultiHeadAttention (N=16, L=1024, E=512, H=8).

Data-parallel over batch: 8 NeuronCores x 2 batches each.

Design:
- Host-side layout prep: q/k/v passed pre-transposed [NB, E, L] fp16 and
  weights pre-transposed [P, EPO, E] fp16 (1/sqrt(D) folded into Wq), so the
  device does no transposes and no input casts.
- S^T logits per head-pair computed as two concurrent K=64 matmuls packed
  into row-groups (tile_position (0,0)/(64,0)) writing halves of one
  [128,1024] fp32 PSUM tile; one N=1024 Exp per tile on ScalarE.
- AV with an appended ones column (M=65) so the softmax denominator falls
  out as PSUM row 64; denominators collected per half-batch, recip via
  Ln+Exp(-x) on ScalarE (same table set as the softmax Exp), broadcast
  across partitions with a selector matmul, normalization as an in-place
  multiply on the staged O^T.
- Q/K projections are emitted per-fpo-block immediately before the S pair
  that consumes them, so the first Exp lands as early as possible; the
  whole schedule is software-pipelined across the two batches and the
  output projection is split into two accumulation rounds so only the
  hpo2/3 half waits on the final normalization.
"""

import numpy as np
import concourse.bass as bass
import concourse.mybir as mybir
import concourse.tile as tile
from contextlib import ExitStack
from concourse import bacc

P = 128
L = 1024
E = 512
H = 8
D = 64
NB = 2            # batches per core
TT = L // P       # 8 token tiles per batch
EPO = E // P      # 4 e-subtiles
CH = L // E       # 2 query chunks of 512
HP = H // 2       # 4 head pairs
F32 = mybir.dt.float32
F16 = mybir.dt.float16
AF = mybir.ActivationFunctionType
ALU = mybir.AluOpType


def build(debug=False):
    nc = bacc.Bacc("TRN2", target_bir_lowering=False, debug=debug)
    qT_d = nc.dram_tensor("qT", [NB, E, L], F16, kind="ExternalInput").ap()
    kT_d = nc.dram_tensor("kT", [NB, E, L], F16, kind="ExternalInput").ap()
    vT_d = nc.dram_tensor("vT", [NB, E, L], F16, kind="ExternalInput").ap()
    wq_d = nc.dram_tensor("wqT", [P, EPO, E], F16, kind="ExternalInput").ap()
    wk_d = nc.dram_tensor("wkT", [P, EPO, E], F16, kind="ExternalInput").ap()
    wv_d = nc.dram_tensor("wvT", [P, EPO, E], F16, kind="ExternalInput").ap()
    wo_d = nc.dram_tensor("woT", [P, EPO, E], F16, kind="ExternalInput").ap()
    sel_d = nc.dram_tensor("sel2", [P, H * D], F16, kind="ExternalInput").ap()
    bo_d = nc.dram_tensor("bo_bcast", [P, E], F32, kind="ExternalInput").ap()
    out_d = nc.dram_tensor("out", [NB, L, E], F32, kind="ExternalOutput").ap()

    with tile.TileContext(nc) as tc, ExitStack() as ctx:
        consts = ctx.enter_context(tc.tile_pool(name="consts", bufs=1))
        xt_pool = ctx.enter_context(tc.tile_pool(name="xt", bufs=4))
        ht_pool = ctx.enter_context(tc.tile_pool(name="ht", bufs=2))
        vh_pool = ctx.enter_context(tc.tile_pool(name="vh", bufs=2))
        pts_pool = ctx.enter_context(tc.tile_pool(name="pts", bufs=20))
        st_pool = ctx.enter_context(tc.tile_pool(name="st", bufs=2))
        rc_pool = ctx.enter_context(tc.tile_pool(name="rc", bufs=2))
        ot_pool = ctx.enter_context(tc.tile_pool(name="ot", bufs=3))
        ps_s = ctx.enter_context(tc.tile_pool(name="pss", bufs=2, space="PSUM"))
        ps_av = ctx.enter_context(tc.tile_pool(name="psav", bufs=2, space="PSUM"))
        ps_x = ctx.enter_context(tc.tile_pool(name="psx", bufs=2, space="PSUM"))

        wts = {}
        xts = [{} for _ in range(NB)]
        qkh = [{} for _ in range(NB)]
        vhs = [None] * NB
        stages = [None] * NB
        denoms = [None] * NB
        recips = [None] * NB
        ptss = [{} for _ in range(NB)]
        oparts = [[None] * TT for _ in range(NB)]

        def load_w(wname, w_d, engs):
            # partition-range chunks: full 4KB/partition descriptors, with
            # chunks spread across queues for parallelism
            wt = consts.tile([P, EPO, E], F16, tag=f"wt_{wname}",
                             name=f"wt_{wname}")
            for i in range(4):
                engs[i % len(engs)].dma_start(
                    wt[i * 32:(i + 1) * 32, :, :], w_d[i * 32:(i + 1) * 32])
            wts[wname] = wt

        def load_x(b, tname, x_d, engs):
            # chunked across several dma_starts: each dma_start's descriptors
            # land on one hw queue, so more chunks = more queue parallelism
            xt = xt_pool.tile([P, EPO, L], F16, tag="xt", name=f"xt_{tname}{b}")
            src = x_d[b].rearrange("(epo p) t -> p epo t", p=P)
            i = 0
            for epo in range(EPO):
                for ph in range(2):
                    engs[i % len(engs)].dma_start(
                        xt[ph * 64:(ph + 1) * 64, epo, :],
                        src[ph * 64:(ph + 1) * 64, epo, :])
                    i += 1
            xts[b][tname] = xt

        def make_ht(b, tname):
            ht = ht_pool.tile([P, EPO, L], F16, tag=f"{tname}ht",
                              name=f"{tname}ht{b}")
            qkh[b][tname] = ht

        def emit_projQK_block(b, tname, fpo, tch):
            wt = wts[tname]
            xt = xts[b][tname]
            ht = qkh[b][tname]
            ps = ps_x.tile([P, E], F32, tag="x", name="ps_qk")
            for epo in range(EPO):
                nc.tensor.matmul(
                    ps[:],
                    wt[:, epo, fpo * P:(fpo + 1) * P],
                    xt[:, epo, tch * E:(tch + 1) * E],
                    start=(epo == 0),
                    stop=(epo == EPO - 1),
                )
            nc.vector.tensor_copy(ht[:, fpo, tch * E:(tch + 1) * E], ps[:])

        def emit_projV(b):
            vh = vh_pool.tile([P, TT, H, D + 1], F16, tag="vh", name=f"vh{b}")
            nc.vector.memset(vh[:], 1.0)  # ones col at [:,:,:,D]
            wt = wts["v"]
            xt = xts[b]["v"]
            for tt in range(TT):
                ps = ps_x.tile([P, E], F32, tag="x", name="ps_v")
                for epo in range(EPO):
                    nc.tensor.matmul(
                        ps[:],
                        xt[:, epo, tt * P:(tt + 1) * P],
                        wt[:, epo, :],
                        start=(epo == 0),
                        stop=(epo == EPO - 1),
                    )
                nc.vector.tensor_copy(
                    vh[:, tt, :, 0:D],
                    ps[:].rearrange("p (h d) -> p h d", h=H),
                )
            vhs[b] = vh

        def attn_state(b):
            stages[b] = st_pool.tile([P, EPO, L], F16, tag="st",
                                     name=f"stage{b}")
            denoms[b] = rc_pool.tile([P, 2, L], F32, tag="dn", name=f"denom{b}")
            nc.vector.memset(denoms[b][:], 1.0)
            recips[b] = rc_pool.tile([P, 2, L], F16, tag="rcp", name=f"recip{b}")

        def emit_s_units(b, hp, units):
            qht, kht = qkh[b]["q"], qkh[b]["k"]
            pts = ptss[b]
            for lt, chq in units:
                ps = ps_s.tile([P, L], F32, tag="s", name="ps_s")
                nc.tensor.matmul(
                    ps[:, 0:E],
                    kht[0:D, hp, lt * P:(lt + 1) * P],
                    qht[0:D, hp, chq * E:(chq + 1) * E],
                    start=True, stop=True,
                    tile_position=(0, 0),
                )
                nc.tensor.matmul(
                    ps[:, E:L],
                    kht[D:P, hp, lt * P:(lt + 1) * P],
                    qht[D:P, hp, chq * E:(chq + 1) * E],
                    start=True, stop=True,
                    tile_position=(D, 0),
                )
                pt = pts_pool.tile([P, L], F16, tag="pts", name="pt")
                nc.scalar.activation(pt[:], ps[:], AF.Exp)
                pts[(hp, lt, chq)] = pt

        def emit_QK_S(b, hp):
            # Q/K projection blocks for this pair's fpo, interleaved so the
            # first S units become ready as early as possible.
            emit_projQK_block(b, "q", hp, 0)
            emit_projQK_block(b, "k", hp, 0)
            emit_s_units(b, hp, [(lt, 0) for lt in range(4)])
            emit_projQK_block(b, "q", hp, 1)
            emit_projQK_block(b, "k", hp, 1)
            emit_s_units(b, hp, [(lt, 1) for lt in range(4)]
                         + [(lt, c) for lt in range(4, 8) for c in range(CH)])

        def emit_av(b, hp):
            vh = vhs[b]
            stage = stages[b]
            denom = denoms[b]
            pts = ptss[b]
            for hi in range(2):
                h = 2 * hp + hi
                hoff = D * hi
                avps = [ps_av.tile([D + 1, E], F32, tag="av", name="ps_av")
                        for _ in range(CH)]
                # alternate chunk banks so consecutive matmuls never
                # accumulate into the same psum bank back-to-back
                for lt in range(TT):
                    for chq in range(CH):
                        nc.tensor.matmul(
                            avps[chq][:],
                            vh[:, lt, h, :],
                            pts[(hp, lt, chq)][:, hi * E:(hi + 1) * E],
                            start=(lt == 0),
                            stop=(lt == TT - 1),
                        )
                for chq in range(CH):
                    avp = avps[chq]
                    nc.vector.tensor_copy(
                        stage[hoff:hoff + D, hp, chq * E:(chq + 1) * E],
                        avp[0:D, :],
                    )
                    nc.vector.tensor_copy(
                        denom[32 * (h % 4):32 * (h % 4) + 1, h // 4,
                              chq * E:(chq + 1) * E],
                        avp[D:D + 1, :],
                    )

        def emit_norm_half(b, half):
            # heads 4*half .. 4*half+3 are collected in denom[:, half, :]
            stage = stages[b]
            nc.scalar.activation(
                denoms[b][:, half, :], denoms[b][:, half, :], AF.Ln)
            nc.scalar.activation(
                recips[b][:, half, :], denoms[b][:, half, :], AF.Exp,
                scale=-1.0)
            sel2 = wts["sel2"]
            for h in range(4 * half, 4 * half + 4):
                hoff = D * (h % 2)
                hpp = h // 2
                for chq in range(CH):
                    # lives in the AV pool so a stalled norm chain never
                    # gates the proj/S/O psum rotation in ps_x
                    psb = ps_av.tile([D + 1, E], F32, tag="av", name="ps_bc")
                    nc.tensor.matmul(
                        psb[0:D, :],
                        sel2[:, h * D:(h + 1) * D],
                        recips[b][:, half, chq * E:(chq + 1) * E],
                        start=True, stop=True,
                    )
                    nc.vector.tensor_tensor(
                        stage[hoff:hoff + D, hpp, chq * E:(chq + 1) * E],
                        psb[0:D, :],
                        stage[hoff:hoff + D, hpp, chq * E:(chq + 1) * E],
                        ALU.mult,
                    )

        def emit_oproj_round(b, rnd):
            wt = wts["o"]
            stage = stages[b]
            for tt in range(TT):
                ps = ps_x.tile([P, E], F32, tag="x", name="ps_o")
                for i, hp in enumerate((0, 1) if rnd == 0 else (2, 3)):
                    nc.tensor.matmul(
                        ps[:],
                        stage[:, hp, tt * P:(tt + 1) * P],
                        wt[:, hp, :],
                        start=(i == 0),
                        stop=(i == 1),
                    )
                if rnd == 0:
                    otp = ot_pool.tile([P, E], F32, tag="otp", bufs=8,
                                       name="otp")
                    nc.vector.tensor_tensor(otp[:], ps[:], bo_t[:], ALU.add)
                    oparts[b][tt] = otp
                else:
                    ot = ot_pool.tile([P, E], F32, tag="ot", name="ot")
                    nc.vector.tensor_tensor(ot[:], ps[:], oparts[b][tt][:],
                                            ALU.add)
                    nc.gpsimd.dma_start(out_d[b, tt * P:(tt + 1) * P, :],
                                        ot[:])

        # ---- emission: software-pipelined across the two batches ----
        # batch-0 first-wave inputs split across all three DMA enqueuers
        # (ScalarE is idle at startup); batch-1 inputs avoid ScalarE.
        load_x(0, "q", qT_d, [nc.sync, nc.scalar, nc.gpsimd])
        load_x(0, "k", kT_d, [nc.scalar, nc.gpsimd, nc.sync])
        load_w("q", wq_d, [nc.gpsimd, nc.sync])
        load_w("k", wk_d, [nc.scalar, nc.gpsimd])
        load_x(0, "v", vT_d, [nc.sync, nc.scalar, nc.gpsimd])
        load_w("v", wv_d, [nc.scalar, nc.sync])
        load_w("o", wo_d, [nc.sync, nc.gpsimd])
        sel2_t = consts.tile([P, H * D], F16, tag="sel2", name="sel2_t")
        nc.sync.dma_start(sel2_t[:], sel_d)
        wts["sel2"] = sel2_t
        bo_t = consts.tile([P, E], F32, tag="bo", name="bo_t")
        nc.sync.dma_start(bo_t[:], bo_d)

        for b in range(NB):
            make_ht(b, "q")
            make_ht(b, "k")

        attn_state(0)
        emit_QK_S(0, 0)
        emit_projV(0)
        emit_av(0, 0)
        emit_QK_S(0, 1)
        emit_av(0, 1)
        emit_norm_half(0, 0)
        emit_QK_S(0, 2)
        load_x(1, "q", qT_d, [nc.sync, nc.gpsimd])
        load_x(1, "k", kT_d, [nc.gpsimd, nc.sync])
        load_x(1, "v", vT_d, [nc.sync, nc.gpsimd])
        emit_av(0, 2)
        attn_state(1)
        emit_QK_S(0, 3)
        emit_QK_S(1, 0)
        emit_av(0, 3)
        emit_norm_half(0, 1)
        emit_projV(1)
        emit_oproj_round(0, 0)
        emit_av(1, 0)
        emit_oproj_round(0, 1)
        emit_QK_S(1, 1)
        emit_av(1, 1)
        emit_norm_half(1, 0)
        emit_QK_S(1, 2)
        emit_av(1, 2)
        emit_oproj_round(1, 0)
        emit_QK_S(1, 3)
        emit_av(1, 3)
        emit_norm_half(1, 1)
        emit_oproj_round(1, 1)

    nc.compile()
    return nc


_COMPILED = None


def _get_compiled():
    global _COMPILED
    if _COMPILED is None:
        _COMPILED = build()
    return _COMPILED


def prepare_in_maps(q, k, v, Wq, Wk, Wv, Wo, bo, n_cores=8):
    """Host-side layout prep shared by kernel() and the test harness."""
    qT = np.ascontiguousarray(
        np.asarray(q, np.float32).transpose(0, 2, 1).astype(np.float16))
    kT = np.ascontiguousarray(
        np.asarray(k, np.float32).transpose(0, 2, 1).astype(np.float16))
    vT = np.ascontiguousarray(
        np.asarray(v, np.float32).transpose(0, 2, 1).astype(np.float16))

    def wprep(W, scale=1.0):
        # wt[p, epo, f] = W[f, epo*128 + p] * scale
        a = (np.asarray(W, np.float32).T * scale).astype(np.float16)  # [e, f]
        return np.ascontiguousarray(a.reshape(EPO, P, E).transpose(1, 0, 2))

    wqT = wprep(Wq, 1.0 / np.sqrt(D))
    wkT = wprep(Wk)
    wvT = wprep(Wv)
    woT = wprep(Wo)
    sel2 = np.zeros((P, H * D), np.float16)
    for h in range(H):
        sel2[32 * (h % 4), h * D:(h + 1) * D] = 1.0
    bo_bcast = np.ascontiguousarray(
        np.broadcast_to(np.asarray(bo, np.float32), (P, E)))

    in_maps = []
    for c in range(n_cores):
        in_maps.append({
            "qT": np.ascontiguousarray(qT[c * NB:(c + 1) * NB]),
            "kT": np.ascontiguousarray(kT[c * NB:(c + 1) * NB]),
            "vT": np.ascontiguousarray(vT[c * NB:(c + 1) * NB]),
            "wqT": wqT, "wkT": wkT, "wvT": wvT, "woT": woT,
            "sel2": sel2, "bo_bcast": bo_bcast,
        })
    return in_maps


def kernel(q, k, v, Wq, Wk, Wv, Wo, bo):
    n_cores = 8
    nc = _get_compiled()
    in_maps = prepare_in_maps(q, k, v, Wq, Wk, Wv, Wo, bo, n_cores)
    from concourse.bass_utils import run_bass_kernel_spmd
    res = run_bass_kernel_spmd(nc, in_maps, core_ids=list(range(n_cores)))
    out = np.concatenate([res.results[c]["out"] for c in range(n_cores)], axis=0)
    return out.astype(np.float32)


# revision 23
# speedup vs baseline: 1.0054x; 1.0054x over previous
"""Trainium2 Bass kernel for ConvMultiHeadAttention (N=16, L=1024, E=512, H=8).

Data-parallel over batch: 8 NeuronCores x 2 batches each.

Design:
- Host-side layout prep: q/k/v passed pre-transposed [NB, E, L] fp16 and
  weights pre-transposed [P, EPO, E] fp16 (1/sqrt(D) folded into Wq), so the
  device does no transposes and no input casts.
- S^T logits per head-pair computed as two concurrent K=64 matmuls packed
  into row-groups (tile_position (0,0)/(64,0)) writing halves of one
  [128,1024] fp32 PSUM tile; one N=1024 Exp per tile on ScalarE.
- AV with an appended ones column (M=65) so the softmax denominator falls
  out as PSUM row 64; denominators collected per half-batch, recip via
  Ln+Exp(-x) on ScalarE (same table set as the softmax Exp), broadcast
  across partitions with a selector matmul, normalization as an in-place
  multiply on the staged O^T.
- Q/K projections are emitted per-fpo-block immediately before the S pair
  that consumes them, so the first Exp lands as early as possible; the
  whole schedule is software-pipelined across the two batches and the
  output projection is split into two accumulation rounds so only the
  hpo2/3 half waits on the final normalization.
"""

import numpy as np
import concourse.bass as bass
import concourse.mybir as mybir
import concourse.tile as tile
from contextlib import ExitStack
from concourse import bacc

P = 128
L = 1024
E = 512
H = 8
D = 64
NB = 2            # batches per core
TT = L // P       # 8 token tiles per batch
EPO = E // P      # 4 e-subtiles
CH = L // E       # 2 query chunks of 512
HP = H // 2       # 4 head pairs
F32 = mybir.dt.float32
F16 = mybir.dt.float16
AF = mybir.ActivationFunctionType
ALU = mybir.AluOpType


def build(debug=False):
    nc = bacc.Bacc("TRN2", target_bir_lowering=False, debug=debug)
    qT_d = nc.dram_tensor("qT", [NB, E, L], F16, kind="ExternalInput").ap()
    kT_d = nc.dram_tensor("kT", [NB, E, L], F16, kind="ExternalInput").ap()
    vT_d = nc.dram_tensor("vT", [NB, E, L], F16, kind="ExternalInput").ap()
    wq_d = nc.dram_tensor("wqT", [P, EPO, E], F16, kind="ExternalInput").ap()
    wk_d = nc.dram_tensor("wkT", [P, EPO, E], F16, kind="ExternalInput").ap()
    wv_d = nc.dram_tensor("wvT", [P, EPO, E], F16, kind="ExternalInput").ap()
    wo_d = nc.dram_tensor("woT", [P, EPO, E], F16, kind="ExternalInput").ap()
    sel_d = nc.dram_tensor("sel2", [P, H * D], F16, kind="ExternalInput").ap()
    bo_d = nc.dram_tensor("bo_bcast", [P, E], F32, kind="ExternalInput").ap()
    out_d = nc.dram_tensor("out", [NB, L, E], F32, kind="ExternalOutput").ap()

    with tile.TileContext(nc) as tc, ExitStack() as ctx:
        consts = ctx.enter_context(tc.tile_pool(name="consts", bufs=1))
        xt_pool = ctx.enter_context(tc.tile_pool(name="xt", bufs=4))
        ht_pool = ctx.enter_context(tc.tile_pool(name="ht", bufs=2))
        vh_pool = ctx.enter_context(tc.tile_pool(name="vh", bufs=2))
        pts_pool = ctx.enter_context(tc.tile_pool(name="pts", bufs=24))
        st_pool = ctx.enter_context(tc.tile_pool(name="st", bufs=2))
        rc_pool = ctx.enter_context(tc.tile_pool(name="rc", bufs=2))
        ot_pool = ctx.enter_context(tc.tile_pool(name="ot", bufs=3))
        ps_s = ctx.enter_context(tc.tile_pool(name="pss", bufs=2, space="PSUM"))
        ps_av = ctx.enter_context(tc.tile_pool(name="psav", bufs=2, space="PSUM"))
        ps_x = ctx.enter_context(tc.tile_pool(name="psx", bufs=2, space="PSUM"))

        wts = {}
        xts = [{} for _ in range(NB)]
        qkh = [{} for _ in range(NB)]
        vhs = [None] * NB
        stages = [None] * NB
        denoms = [None] * NB
        recips = [None] * NB
        ptss = [{} for _ in range(NB)]
        oparts = [[None] * TT for _ in range(NB)]

        def load_w(wname, w_d, engs):
            # partition-range chunks: full 4KB/partition descriptors, with
            # chunks spread across queues for parallelism
            wt = consts.tile([P, EPO, E], F16, tag=f"wt_{wname}",
                             name=f"wt_{wname}")
            for i in range(4):
                engs[i % len(engs)].dma_start(
                    wt[i * 32:(i + 1) * 32, :, :], w_d[i * 32:(i + 1) * 32])
            wts[wname] = wt

        def load_x(b, tname, x_d, engs):
            # chunked across several dma_starts: each dma_start's descriptors
            # land on one hw queue, so more chunks = more queue parallelism
            xt = xt_pool.tile([P, EPO, L], F16, tag="xt", name=f"xt_{tname}{b}")
            src = x_d[b].rearrange("(epo p) t -> p epo t", p=P)
            i = 0
            for epo in range(EPO):
                for ph in range(2):
                    engs[i % len(engs)].dma_start(
                        xt[ph * 64:(ph + 1) * 64, epo, :],
                        src[ph * 64:(ph + 1) * 64, epo, :])
                    i += 1
            xts[b][tname] = xt

        def make_ht(b, tname):
            ht = ht_pool.tile([P, EPO, L], F16, tag=f"{tname}ht",
                              name=f"{tname}ht{b}")
            qkh[b][tname] = ht

        def emit_projQK_block(b, tname, fpo, tch):
            wt = wts[tname]
            xt = xts[b][tname]
            ht = qkh[b][tname]
            ps = ps_x.tile([P, E], F32, tag="x", name="ps_qk")
            for epo in range(EPO):
                nc.tensor.matmul(
                    ps[:],
                    wt[:, epo, fpo * P:(fpo + 1) * P],
                    xt[:, epo, tch * E:(tch + 1) * E],
                    start=(epo == 0),
                    stop=(epo == EPO - 1),
                )
            nc.vector.tensor_copy(ht[:, fpo, tch * E:(tch + 1) * E], ps[:])

        def emit_projV(b):
            vh = vh_pool.tile([P, TT, H, D + 1], F16, tag="vh", name=f"vh{b}")
            nc.vector.memset(vh[:], 1.0)  # ones col at [:,:,:,D]
            wt = wts["v"]
            xt = xts[b]["v"]
            for tt in range(TT):
                ps = ps_x.tile([P, E], F32, tag="x", name="ps_v")
                for epo in range(EPO):
                    nc.tensor.matmul(
                        ps[:],
                        xt[:, epo, tt * P:(tt + 1) * P],
                        wt[:, epo, :],
                        start=(epo == 0),
                        stop=(epo == EPO - 1),
                    )
                nc.vector.tensor_copy(
                    vh[:, tt, :, 0:D],
                    ps[:].rearrange("p (h d) -> p h d", h=H),
                )
            vhs[b] = vh

        def attn_state(b):
            stages[b] = st_pool.tile([P, EPO, L], F16, tag="st",
                                     name=f"stage{b}")
            denoms[b] = rc_pool.tile([P, 2, L], F32, tag="dn", name=f"denom{b}")
            nc.vector.memset(denoms[b][:], 1.0)
            recips[b] = rc_pool.tile([P, 2, L], F16, tag="rcp", name=f"recip{b}")

        def emit_s_units(b, hp, units):
            qht, kht = qkh[b]["q"], qkh[b]["k"]
            pts = ptss[b]
            for lt, chq in units:
                ps = ps_s.tile([P, L], F32, tag="s", name="ps_s")
                nc.tensor.matmul(
                    ps[:, 0:E],
                    kht[0:D, hp, lt * P:(lt + 1) * P],
                    qht[0:D, hp, chq * E:(chq + 1) * E],
                    start=True, stop=True,
                    tile_position=(0, 0),
                )
                nc.tensor.matmul(
                    ps[:, E:L],
                    kht[D:P, hp, lt * P:(lt + 1) * P],
                    qht[D:P, hp, chq * E:(chq + 1) * E],
                    start=True, stop=True,
                    tile_position=(D, 0),
                )
                pt = pts_pool.tile([P, L], F16, tag="pts", name="pt")
                nc.scalar.activation(pt[:], ps[:], AF.Exp)
                pts[(hp, lt, chq)] = pt

        def emit_QK_pair(b, hp):
            for tch in range(CH):
                emit_projQK_block(b, "q", hp, tch)
                emit_projQK_block(b, "k", hp, tch)

        def emit_S_pair(b, hp):
            emit_s_units(b, hp,
                         [(lt, c) for lt in range(TT) for c in range(CH)])

        def emit_QK_S(b, hp):
            # Q/K projection blocks for this pair's fpo, interleaved so the
            # first S units become ready as early as possible.
            emit_projQK_block(b, "q", hp, 0)
            emit_projQK_block(b, "k", hp, 0)
            emit_s_units(b, hp, [(lt, 0) for lt in range(4)])
            emit_projQK_block(b, "q", hp, 1)
            emit_projQK_block(b, "k", hp, 1)
            emit_s_units(b, hp, [(lt, 1) for lt in range(4)]
                         + [(lt, c) for lt in range(4, 8) for c in range(CH)])

        def emit_av(b, hp):
            vh = vhs[b]
            stage = stages[b]
            denom = denoms[b]
            pts = ptss[b]
            for hi in range(2):
                h = 2 * hp + hi
                hoff = D * hi
                avps = [ps_av.tile([D + 1, E], F32, tag="av", name="ps_av")
                        for _ in range(CH)]
                # alternate chunk banks so consecutive matmuls never
                # accumulate into the same psum bank back-to-back
                for lt in range(TT):
                    for chq in range(CH):
                        nc.tensor.matmul(
                            avps[chq][:],
                            vh[:, lt, h, :],
                            pts[(hp, lt, chq)][:, hi * E:(hi + 1) * E],
                            start=(lt == 0),
                            stop=(lt == TT - 1),
                        )
                for chq in range(CH):
                    avp = avps[chq]
                    nc.vector.tensor_copy(
                        stage[hoff:hoff + D, hp, chq * E:(chq + 1) * E],
                        avp[0:D, :],
                    )
                    nc.vector.tensor_copy(
                        denom[32 * (h % 4):32 * (h % 4) + 1, h // 4,
                              chq * E:(chq + 1) * E],
                        avp[D:D + 1, :],
                    )

        def emit_norm_half(b, half):
            # heads 4*half .. 4*half+3 are collected in denom[:, half, :]
            stage = stages[b]
            nc.scalar.activation(
                denoms[b][:, half, :], denoms[b][:, half, :], AF.Ln)
            nc.scalar.activation(
                recips[b][:, half, :], denoms[b][:, half, :], AF.Exp,
                scale=-1.0)
            sel2 = wts["sel2"]
            for h in range(4 * half, 4 * half + 4):
                hoff = D * (h % 2)
                hpp = h // 2
                for chq in range(CH):
                    # lives in the AV pool so a stalled norm chain never
                    # gates the proj/S/O psum rotation in ps_x
                    psb = ps_av.tile([D + 1, E], F32, tag="av", name="ps_bc")
                    nc.tensor.matmul(
                        psb[0:D, :],
                        sel2[:, h * D:(h + 1) * D],
                        recips[b][:, half, chq * E:(chq + 1) * E],
                        start=True, stop=True,
                    )
                    nc.vector.tensor_tensor(
                        stage[hoff:hoff + D, hpp, chq * E:(chq + 1) * E],
                        psb[0:D, :],
                        stage[hoff:hoff + D, hpp, chq * E:(chq + 1) * E],
                        ALU.mult,
                    )

        def emit_oproj_round(b, rnd):
            wt = wts["o"]
            stage = stages[b]
            for tt in range(TT):
                ps = ps_x.tile([P, E], F32, tag="x", name="ps_o")
                for i, hp in enumerate((0, 1) if rnd == 0 else (2, 3)):
                    nc.tensor.matmul(
                        ps[:],
                        stage[:, hp, tt * P:(tt + 1) * P],
                        wt[:, hp, :],
                        start=(i == 0),
                        stop=(i == 1),
                    )
                if rnd == 0:
                    otp = ot_pool.tile([P, E], F16, tag="otp", bufs=8,
                                       name="otp")
                    nc.vector.tensor_tensor(otp[:], ps[:], bo_t[:], ALU.add)
                    oparts[b][tt] = otp
                else:
                    ot = ot_pool.tile([P, E], F32, tag="ot", name="ot")
                    nc.vector.tensor_tensor(ot[:], ps[:], oparts[b][tt][:],
                                            ALU.add)
                    nc.gpsimd.dma_start(out_d[b, tt * P:(tt + 1) * P, :],
                                        ot[:])

        # ---- emission: software-pipelined across the two batches ----
        # First-wave inputs (q/k/wq/wk) get the DMA queues to themselves;
        # everything else is enqueued after pair-0 compute is emitted.
        load_x(0, "q", qT_d, [nc.sync, nc.scalar])
        load_x(0, "k", kT_d, [nc.scalar, nc.sync])
        load_w("q", wq_d, [nc.gpsimd, nc.sync])
        load_w("k", wk_d, [nc.scalar, nc.gpsimd])

        for b in range(NB):
            make_ht(b, "q")
            make_ht(b, "k")

        attn_state(0)
        emit_QK_S(0, 0)
        load_x(0, "v", vT_d, [nc.sync, nc.scalar, nc.gpsimd])
        load_w("v", wv_d, [nc.scalar, nc.sync])
        sel2_t = consts.tile([P, H * D], F16, tag="sel2", name="sel2_t")
        nc.gpsimd.dma_start(sel2_t[:], sel_d)
        wts["sel2"] = sel2_t
        bo_t = consts.tile([P, E], F32, tag="bo", name="bo_t")
        nc.gpsimd.dma_start(bo_t[:], bo_d)
        emit_projV(0)
        load_w("o", wo_d, [nc.sync, nc.gpsimd])
        emit_av(0, 0)
        emit_QK_S(0, 1)
        emit_av(0, 1)
        emit_norm_half(0, 0)
        emit_QK_S(0, 2)
        load_x(1, "q", qT_d, [nc.sync, nc.gpsimd])
        load_x(1, "k", kT_d, [nc.gpsimd, nc.sync])
        load_x(1, "v", vT_d, [nc.sync, nc.gpsimd])
        emit_av(0, 2)
        attn_state(1)
        emit_QK_pair(1, 0)
        emit_QK_S(0, 3)
        emit_av(0, 3)
        emit_norm_half(0, 1)
        emit_S_pair(1, 0)
        emit_projV(1)
        emit_oproj_round(0, 0)
        emit_av(1, 0)
        emit_oproj_round(0, 1)
        emit_QK_S(1, 1)
        emit_av(1, 1)
        emit_norm_half(1, 0)
        emit_QK_S(1, 2)
        emit_av(1, 2)
        emit_oproj_round(1, 0)
        emit_QK_S(1, 3)
        emit_av(1, 3)
        emit_norm_half(1, 1)
        emit_oproj_round(1, 1)

    nc.compile()
    return nc


_COMPILED = None


def _get_compiled():
    global _COMPILED
    if _COMPILED is None:
        _COMPILED = build()
    return _COMPILED


def prepare_in_maps(q, k, v, Wq, Wk, Wv, Wo, bo, n_cores=8):
    """Host-side layout prep shared by kernel() and the test harness."""
    qT = np.ascontiguousarray(
        np.asarray(q, np.float32).transpose(0, 2, 1).astype(np.float16))
    kT = np.ascontiguousarray(
        np.asarray(k, np.float32).transpose(0, 2, 1).astype(np.float16))
    vT = np.ascontiguousarray(
        np.asarray(v, np.float32).transpose(0, 2, 1).astype(np.float16))

    def wprep(W, scale=1.0):
        # wt[p, epo, f] = W[f, epo*128 + p] * scale
        a = (np.asarray(W, np.float32).T * scale).astype(np.float16)  # [e, f]
        return np.ascontiguousarray(a.reshape(EPO, P, E).transpose(1, 0, 2))

    wqT = wprep(Wq, 1.0 / np.sqrt(D))
    wkT = wprep(Wk)
    wvT = wprep(Wv)
    woT = wprep(Wo)
    sel2 = np.zeros((P, H * D), np.float16)
    for h in range(H):
        sel2[32 * (h % 4), h * D:(h + 1) * D] = 1.0
    bo_bcast = np.ascontiguousarray(
        np.broadcast_to(np.asarray(bo, np.float32), (P, E)))

    in_maps = []
    for c in range(n_cores):
        in_maps.append({
            "qT": np.ascontiguousarray(qT[c * NB:(c + 1) * NB]),
            "kT": np.ascontiguousarray(kT[c * NB:(c + 1) * NB]),
            "vT": np.ascontiguousarray(vT[c * NB:(c + 1) * NB]),
            "wqT": wqT, "wkT": wkT, "wvT": wvT, "woT": woT,
            "sel2": sel2, "bo_bcast": bo_bcast,
        })
    return in_maps


def kernel(q, k, v, Wq, Wk, Wv, Wo, bo):
    n_cores = 8
    nc = _get_compiled()
    in_maps = prepare_in_maps(q, k, v, Wq, Wk, Wv, Wo, bo, n_cores)
    from concourse.bass_utils import run_bass_kernel_spmd
    res = run_bass_kernel_spmd(nc, in_maps, core_ids=list(range(n_cores)))
    out = np.concatenate([res.results[c]["out"] for c in range(n_cores)], axis=0)
    return out.astype(np.float32)


# revision 28
# speedup vs baseline: 1.0801x; 1.0743x over previous
"""Trainium2 Bass kernel for ConvMultiHeadAttention (N=16, L=1024, E=512, H=8).

Data-parallel over batch: 8 NeuronCores x 2 batches each.

Design:
- Host-side layout prep: q/k/v passed pre-transposed [NB, E, L] fp16 and
  weights pre-transposed [P, EPO, E] fp16 (1/sqrt(D) folded into Wq), so the
  device does no transposes and no input casts.
- S^T logits per head-pair computed as two concurrent K=64 matmuls packed
  into row-groups (tile_position (0,0)/(64,0)) writing halves of one
  [128,1024] fp32 PSUM tile; one N=1024 Exp per tile on ScalarE.
- AV with an appended ones column (M=65) so the softmax denominator falls
  out as PSUM row 64; denominators collected per half-batch, recip via
  Ln+Exp(-x) on ScalarE (same table set as the softmax Exp), broadcast
  across partitions with a selector matmul, normalization as an in-place
  multiply on the staged O^T.
- Q/K projections are emitted per-fpo-block immediately before the S pair
  that consumes them, so the first Exp lands as early as possible; the
  whole schedule is software-pipelined across the two batches and the
  output projection is split into two accumulation rounds so only the
  hpo2/3 half waits on the final normalization.
"""

import numpy as np
import concourse.bass as bass
import concourse.mybir as mybir
import concourse.tile as tile
from contextlib import ExitStack
from concourse import bacc

P = 128
L = 1024
E = 512
H = 8
D = 64
NB = 2            # batches per core
TT = L // P       # 8 token tiles per batch
EPO = E // P      # 4 e-subtiles
CH = L // E       # 2 query chunks of 512
HP = H // 2       # 4 head pairs
F32 = mybir.dt.float32
F16 = mybir.dt.float16
AF = mybir.ActivationFunctionType
ALU = mybir.AluOpType


def _patch_act_tables():
    """Resolve both Exp and Ln to the one table set that contains both, so
    the kernel never thrashes ACT table loads (each switch costs ~2.7us).
    Order-preserving filter: set ids stay aligned with act_info.json."""
    import concourse.bacc as bacc_mod
    import concourse.hw_specs as hw
    if getattr(bacc_mod, "_act_tables_patched", False):
        return
    orig = hw.get_activation_tables

    def patched(arch):
        out = {}
        for name, fns in orig(arch).items():
            if name != "natural_log_exp_and_others":
                fns = fns - {AF.Exp, AF.Ln}
            out[name] = fns
        return out

    bacc_mod.get_activation_tables = patched
    bacc_mod._act_tables_patched = True


def build(debug=False):
    _patch_act_tables()
    nc = bacc.Bacc("TRN2", target_bir_lowering=False, debug=debug)
    qT_d = nc.dram_tensor("qT", [NB, E, L], F16, kind="ExternalInput").ap()
    kT_d = nc.dram_tensor("kT", [NB, E, L], F16, kind="ExternalInput").ap()
    vT_d = nc.dram_tensor("vT", [NB, E, L], F16, kind="ExternalInput").ap()
    wq_d = nc.dram_tensor("wqT", [P, EPO, E], F16, kind="ExternalInput").ap()
    wk_d = nc.dram_tensor("wkT", [P, EPO, E], F16, kind="ExternalInput").ap()
    wv_d = nc.dram_tensor("wvT", [P, EPO, E], F16, kind="ExternalInput").ap()
    wo_d = nc.dram_tensor("woT", [P, EPO, E], F16, kind="ExternalInput").ap()
    sel_d = nc.dram_tensor("sel2", [P, H * D], F16, kind="ExternalInput").ap()
    bo_d = nc.dram_tensor("bo_bcast", [P, E], F32, kind="ExternalInput").ap()
    out_d = nc.dram_tensor("out", [NB, L, E], F32, kind="ExternalOutput").ap()

    with tile.TileContext(nc) as tc, ExitStack() as ctx:
        consts = ctx.enter_context(tc.tile_pool(name="consts", bufs=1))
        xt_pool = ctx.enter_context(tc.tile_pool(name="xt", bufs=4))
        ht_pool = ctx.enter_context(tc.tile_pool(name="ht", bufs=2))
        vh_pool = ctx.enter_context(tc.tile_pool(name="vh", bufs=2))
        pts_pool = ctx.enter_context(tc.tile_pool(name="pts", bufs=24))
        st_pool = ctx.enter_context(tc.tile_pool(name="st", bufs=2))
        rc_pool = ctx.enter_context(tc.tile_pool(name="rc", bufs=2))
        ot_pool = ctx.enter_context(tc.tile_pool(name="ot", bufs=3))
        ps_s = ctx.enter_context(tc.tile_pool(name="pss", bufs=2, space="PSUM"))
        ps_av = ctx.enter_context(tc.tile_pool(name="psav", bufs=2, space="PSUM"))
        ps_x = ctx.enter_context(tc.tile_pool(name="psx", bufs=2, space="PSUM"))

        wts = {}
        xts = [{} for _ in range(NB)]
        qkh = [{} for _ in range(NB)]
        vhs = [None] * NB
        stages = [None] * NB
        denoms = [None] * NB
        recips = [None] * NB
        ptss = [{} for _ in range(NB)]
        oparts = [[None] * TT for _ in range(NB)]

        def load_w(wname, w_d, engs):
            # partition-range chunks: full 4KB/partition descriptors, with
            # chunks spread across queues for parallelism
            wt = consts.tile([P, EPO, E], F16, tag=f"wt_{wname}",
                             name=f"wt_{wname}")
            for i in range(4):
                engs[i % len(engs)].dma_start(
                    wt[i * 32:(i + 1) * 32, :, :], w_d[i * 32:(i + 1) * 32])
            wts[wname] = wt

        def load_x(b, tname, x_d, engs):
            # chunked across several dma_starts: each dma_start's descriptors
            # land on one hw queue, so more chunks = more queue parallelism
            xt = xt_pool.tile([P, EPO, L], F16, tag="xt", name=f"xt_{tname}{b}")
            src = x_d[b].rearrange("(epo p) t -> p epo t", p=P)
            i = 0
            for epo in range(EPO):
                for ph in range(2):
                    engs[i % len(engs)].dma_start(
                        xt[ph * 64:(ph + 1) * 64, epo, :],
                        src[ph * 64:(ph + 1) * 64, epo, :])
                    i += 1
            xts[b][tname] = xt

        def make_ht(b, tname):
            ht = ht_pool.tile([P, EPO, L], F16, tag=f"{tname}ht",
                              name=f"{tname}ht{b}")
            qkh[b][tname] = ht

        def emit_projQK_block(b, tname, fpo, tch):
            wt = wts[tname]
            xt = xts[b][tname]
            ht = qkh[b][tname]
            ps = ps_x.tile([P, E], F32, tag="x", name="ps_qk")
            for epo in range(EPO):
                nc.tensor.matmul(
                    ps[:],
                    wt[:, epo, fpo * P:(fpo + 1) * P],
                    xt[:, epo, tch * E:(tch + 1) * E],
                    start=(epo == 0),
                    stop=(epo == EPO - 1),
                )
            nc.vector.tensor_copy(ht[:, fpo, tch * E:(tch + 1) * E], ps[:])

        def emit_projV(b):
            vh = vh_pool.tile([P, TT, H, D + 1], F16, tag="vh", name=f"vh{b}")
            nc.vector.memset(vh[:], 1.0)  # ones col at [:,:,:,D]
            wt = wts["v"]
            xt = xts[b]["v"]
            for tt in range(TT):
                ps = ps_x.tile([P, E], F32, tag="x", name="ps_v")
                for epo in range(EPO):
                    nc.tensor.matmul(
                        ps[:],
                        xt[:, epo, tt * P:(tt + 1) * P],
                        wt[:, epo, :],
                        start=(epo == 0),
                        stop=(epo == EPO - 1),
                    )
                nc.vector.tensor_copy(
                    vh[:, tt, :, 0:D],
                    ps[:].rearrange("p (h d) -> p h d", h=H),
                )
            vhs[b] = vh

        def attn_state(b):
            stages[b] = st_pool.tile([P, EPO, L], F16, tag="st",
                                     name=f"stage{b}")
            denoms[b] = rc_pool.tile([P, 2, L], F32, tag="dn", name=f"denom{b}")
            nc.vector.memset(denoms[b][:], 1.0)
            recips[b] = rc_pool.tile([P, 2, L], F16, tag="rcp", name=f"recip{b}")

        def emit_s_units(b, hp, units):
            qht, kht = qkh[b]["q"], qkh[b]["k"]
            pts = ptss[b]
            for lt, chq in units:
                ps = ps_s.tile([P, L], F32, tag="s", name="ps_s")
                nc.tensor.matmul(
                    ps[:, 0:E],
                    kht[0:D, hp, lt * P:(lt + 1) * P],
                    qht[0:D, hp, chq * E:(chq + 1) * E],
                    start=True, stop=True,
                    tile_position=(0, 0),
                )
                nc.tensor.matmul(
                    ps[:, E:L],
                    kht[D:P, hp, lt * P:(lt + 1) * P],
                    qht[D:P, hp, chq * E:(chq + 1) * E],
                    start=True, stop=True,
                    tile_position=(D, 0),
                )
                pt = pts_pool.tile([P, L], F16, tag="pts", name="pt")
                nc.scalar.activation(pt[:], ps[:], AF.Exp)
                pts[(hp, lt, chq)] = pt

        def emit_QK_pair(b, hp):
            for tch in range(CH):
                emit_projQK_block(b, "q", hp, tch)
                emit_projQK_block(b, "k", hp, tch)

        def emit_S_pair(b, hp):
            emit_s_units(b, hp,
                         [(lt, c) for lt in range(TT) for c in range(CH)])

        def emit_QK_S(b, hp):
            # Q/K projection blocks for this pair's fpo, interleaved so the
            # first S units become ready as early as possible.
            emit_projQK_block(b, "q", hp, 0)
            emit_projQK_block(b, "k", hp, 0)
            emit_s_units(b, hp, [(lt, 0) for lt in range(4)])
            emit_projQK_block(b, "q", hp, 1)
            emit_projQK_block(b, "k", hp, 1)
            emit_s_units(b, hp, [(lt, 1) for lt in range(4)]
                         + [(lt, c) for lt in range(4, 8) for c in range(CH)])

        def emit_av(b, hp):
            vh = vhs[b]
            stage = stages[b]
            denom = denoms[b]
            pts = ptss[b]
            for hi in range(2):
                h = 2 * hp + hi
                hoff = D * hi
                avps = [ps_av.tile([D + 1, E], F32, tag="av", name="ps_av")
                        for _ in range(CH)]
                # alternate chunk banks so consecutive matmuls never
                # accumulate into the same psum bank back-to-back
                for lt in range(TT):
                    for chq in range(CH):
                        nc.tensor.matmul(
                            avps[chq][:],
                            vh[:, lt, h, :],
                            pts[(hp, lt, chq)][:, hi * E:(hi + 1) * E],
                            start=(lt == 0),
                            stop=(lt == TT - 1),
                        )
                for chq in range(CH):
                    avp = avps[chq]
                    nc.vector.tensor_copy(
                        stage[hoff:hoff + D, hp, chq * E:(chq + 1) * E],
                        avp[0:D, :],
                    )
                    nc.vector.tensor_copy(
                        denom[32 * (h % 4):32 * (h % 4) + 1, h // 4,
                              chq * E:(chq + 1) * E],
                        avp[D:D + 1, :],
                    )

        def _norm_heads(b, heads, half, pslice):
            # Ln then Exp(-x) over the collected denominators (one table
            # set for both), then per-(head, chunk) partition-broadcast via
            # a selector matmul and in-place multiply on the staged O^T.
            stage = stages[b]
            nc.scalar.activation(
                denoms[b][pslice, half, :], denoms[b][pslice, half, :], AF.Ln)
            nc.scalar.activation(
                recips[b][pslice, half, :], denoms[b][pslice, half, :], AF.Exp,
                scale=-1.0)
            sel2 = wts["sel2"]
            for h in heads:
                hoff = D * (h % 2)
                hpp = h // 2
                for chq in range(CH):
                    # lives in the AV pool so a stalled norm chain never
                    # gates the proj/S/O psum rotation in ps_x
                    psb = ps_av.tile([D + 1, E], F32, tag="av", name="ps_bc")
                    nc.tensor.matmul(
                        psb[0:D, :],
                        sel2[pslice, h * D:(h + 1) * D],
                        recips[b][pslice, half, chq * E:(chq + 1) * E],
                        start=True, stop=True,
                    )
                    nc.vector.tensor_tensor(
                        stage[hoff:hoff + D, hpp, chq * E:(chq + 1) * E],
                        psb[0:D, :],
                        stage[hoff:hoff + D, hpp, chq * E:(chq + 1) * E],
                        ALU.mult,
                    )

        def emit_norm_half(b, half):
            # heads 4*half .. 4*half+3 are collected in denom[:, half, :]
            _norm_heads(b, range(4 * half, 4 * half + 4), half, slice(0, P))

        def emit_norm_pair(b, hp):
            # single head pair: heads 2hp, 2hp+1 live in rows 64*(hp%2)
            # .. +64 of denom bank hp//2 (finer tail granularity)
            ps0 = 64 * (hp % 2)
            _norm_heads(b, (2 * hp, 2 * hp + 1), hp // 2,
                        slice(ps0, ps0 + 64))

        def emit_oproj_round(b, rnd):
            wt = wts["o"]
            stage = stages[b]
            for tt in range(TT):
                ps = ps_x.tile([P, E], F32, tag="x", name="ps_o")
                for i, hp in enumerate((0, 1) if rnd == 0 else (2, 3)):
                    nc.tensor.matmul(
                        ps[:],
                        stage[:, hp, tt * P:(tt + 1) * P],
                        wt[:, hp, :],
                        start=(i == 0),
                        stop=(i == 1),
                    )
                if rnd == 0:
                    otp = ot_pool.tile([P, E], F16, tag="otp", bufs=8,
                                       name="otp")
                    nc.vector.tensor_tensor(otp[:], ps[:], bo_t[:], ALU.add)
                    oparts[b][tt] = otp
                else:
                    ot = ot_pool.tile([P, E], F32, tag="ot", name="ot")
                    nc.vector.tensor_tensor(ot[:], ps[:], oparts[b][tt][:],
                                            ALU.add)
                    # split across queues so the tail drains fast
                    nc.gpsimd.dma_start(
                        out_d[b, tt * P:tt * P + 64, :], ot[0:64, :])
                    nc.sync.dma_start(
                        out_d[b, tt * P + 64:(tt + 1) * P, :], ot[64:P, :])

        # ---- emission: software-pipelined across the two batches ----
        # First-wave inputs (q/k/wq/wk) get the DMA queues to themselves;
        # everything else is enqueued after pair-0 compute is emitted.
        load_x(0, "q", qT_d, [nc.sync, nc.scalar])
        load_x(0, "k", kT_d, [nc.scalar, nc.sync])
        load_w("q", wq_d, [nc.sync, nc.scalar])
        load_w("k", wk_d, [nc.scalar, nc.sync])

        for b in range(NB):
            make_ht(b, "q")
            make_ht(b, "k")

        attn_state(0)
        emit_QK_S(0, 0)
        load_x(0, "v", vT_d, [nc.sync, nc.scalar, nc.gpsimd])
        load_w("v", wv_d, [nc.scalar, nc.sync])
        sel2_t = consts.tile([P, H * D], F16, tag="sel2", name="sel2_t")
        nc.gpsimd.dma_start(sel2_t[:], sel_d)
        wts["sel2"] = sel2_t
        bo_t = consts.tile([P, E], F32, tag="bo", name="bo_t")
        nc.gpsimd.dma_start(bo_t[:], bo_d)
        emit_projV(0)
        load_w("o", wo_d, [nc.sync, nc.gpsimd])
        emit_av(0, 0)
        emit_QK_S(0, 1)
        emit_av(0, 1)
        emit_norm_half(0, 0)
        emit_QK_S(0, 2)
        load_x(1, "q", qT_d, [nc.sync, nc.gpsimd])
        load_x(1, "k", kT_d, [nc.gpsimd, nc.sync])
        load_x(1, "v", vT_d, [nc.sync, nc.gpsimd])
        emit_av(0, 2)
        attn_state(1)
        emit_QK_pair(1, 0)
        emit_QK_S(0, 3)
        emit_av(0, 3)
        emit_norm_half(0, 1)
        emit_S_pair(1, 0)
        emit_projV(1)
        emit_oproj_round(0, 0)
        emit_av(1, 0)
        emit_oproj_round(0, 1)
        emit_QK_S(1, 1)
        emit_av(1, 1)
        emit_norm_half(1, 0)
        emit_QK_S(1, 2)
        emit_av(1, 2)
        emit_oproj_round(1, 0)
        emit_QK_S(1, 3)
        emit_norm_pair(1, 2)
        emit_av(1, 3)
        emit_norm_pair(1, 3)
        emit_oproj_round(1, 1)

    nc.compile()
    return nc


_COMPILED = None


def _get_compiled():
    global _COMPILED
    if _COMPILED is None:
        _COMPILED = build()
    return _COMPILED


def prepare_in_maps(q, k, v, Wq, Wk, Wv, Wo, bo, n_cores=8):
    """Host-side layout prep shared by kernel() and the test harness."""
    qT = np.ascontiguousarray(
        np.asarray(q, np.float32).transpose(0, 2, 1).astype(np.float16))
    kT = np.ascontiguousarray(
        np.asarray(k, np.float32).transpose(0, 2, 1).astype(np.float16))
    vT = np.ascontiguousarray(
        np.asarray(v, np.float32).transpose(0, 2, 1).astype(np.float16))

    def wprep(W, scale=1.0):
        # wt[p, epo, f] = W[f, epo*128 + p] * scale
        a = (np.asarray(W, np.float32).T * scale).astype(np.float16)  # [e, f]
        return np.ascontiguousarray(a.reshape(EPO, P, E).transpose(1, 0, 2))

    wqT = wprep(Wq, 1.0 / np.sqrt(D))
    wkT = wprep(Wk)
    wvT = wprep(Wv)
    woT = wprep(Wo)
    sel2 = np.zeros((P, H * D), np.float16)
    for h in range(H):
        sel2[32 * (h % 4), h * D:(h + 1) * D] = 1.0
    bo_bcast = np.ascontiguousarray(
        np.broadcast_to(np.asarray(bo, np.float32), (P, E)))

    in_maps = []
    for c in range(n_cores):
        in_maps.append({
            "qT": np.ascontiguousarray(qT[c * NB:(c + 1) * NB]),
            "kT": np.ascontiguousarray(kT[c * NB:(c + 1) * NB]),
            "vT": np.ascontiguousarray(vT[c * NB:(c + 1) * NB]),
            "wqT": wqT, "wkT": wkT, "wvT": wvT, "woT": woT,
            "sel2": sel2, "bo_bcast": bo_bcast,
        })
    return in_maps


def kernel(q, k, v, Wq, Wk, Wv, Wo, bo):
    n_cores = 8
    nc = _get_compiled()
    in_maps = prepare_in_maps(q, k, v, Wq, Wk, Wv, Wo, bo, n_cores)
    from concourse.bass_utils import run_bass_kernel_spmd
    res = run_bass_kernel_spmd(nc, in_maps, core_ids=list(range(n_cores)))
    out = np.concatenate([res.results[c]["out"] for c in range(n_cores)], axis=0)
    return out.astype(np.float32)
